# revision 20
# baseline (speedup 1.0000x reference)
"""CPAttention Trainium2 kernel: 8-way batch-data-parallel over 8 NeuronCores.

v2: head-pair processing with PE packing.
  - dots: fp32, two heads row-packed (K=64 at tile_position (0,0)/(64,0))
  - AV:   bf16, two heads col-packed into one [128,1024] PSUM (cols 0:64/64:128)
  - pack: 4-col-packed ones-matmuls -> score_A(row0, fp32), score_B(row32, fp32),
          Z_A(row64, bf16), Z_B(row96, bf16), accumulated over j-tiles
  - outproj: per-pair K=128 bf16
Score path (argsort-critical) stays fp32; softmax/output path is bf16.
Host applies the argsort + 16-step row swap (commutes with w_out).
"""
import numpy as np

import concourse.bacc as bacc
import concourse.tile as tile
from concourse import mybir
from concourse.bass_utils import run_bass_kernel_spmd

F32 = mybir.dt.float32
BF16 = mybir.dt.bfloat16
U32 = mybir.dt.uint32
AOP = mybir.AluOpType
AFT = mybir.ActivationFunctionType

B, N, DIM = 8, 1024, 512
HEADS, DH = 8, 64
INNER = 512
SCALE = DH ** -0.5

_cache = {}


def _build():
    nc = bacc.Bacc()
    xT = nc.declare_dram_parameter("xT", [DIM, N], F32, isOutput=False)
    xTbf = nc.declare_dram_parameter("xTbf", [DIM, N], BF16, isOutput=False)
    maskT = nc.declare_dram_parameter("maskT", [N, N], BF16, isOutput=False)
    wqk = nc.declare_dram_parameter("wqk", [DIM, 2 * INNER], F32, isOutput=False)
    wvbf = nc.declare_dram_parameter("wvbf", [DIM, INNER], BF16, isOutput=False)
    wobf = nc.declare_dram_parameter("wobf", [INNER, DIM], BF16, isOutput=False)
    bout = nc.declare_dram_parameter("bout", [1, DIM], F32, isOutput=False)
    y_out = nc.declare_dram_parameter("y", [N, DIM], F32, isOutput=True)
    sc_out = nc.declare_dram_parameter("score", [1, N], F32, isOutput=True)

    with tile.TileContext(nc) as tc:
        with tc.tile_pool(name="cst", bufs=1) as cst, \
             tc.tile_pool(name="wrk", bufs=3) as wrk, \
             tc.tile_pool(name="wrk4", bufs=4) as wrk4, \
             tc.tile_pool(name="eph", bufs=2) as eph, \
             tc.tile_pool(name="one", bufs=1) as one, \
             tc.tile_pool(name="ppA", bufs=1, space="PSUM") as ppA, \
             tc.tile_pool(name="ppB", bufs=1, space="PSUM") as ppB, \
             tc.tile_pool(name="poT", bufs=1, space="PSUM") as poT, \
             tc.tile_pool(name="ppk", bufs=1, space="PSUM") as ppk:

            # ---- loads ----
            xt_t = []
            wq_t = []
            for kt in range(4):
                xk = cst.tile([128, N], F32, tag=f"xt{kt}")
                nc.sync.dma_start(out=xk, in_=xT[kt * 128:(kt + 1) * 128, :])
                wk = cst.tile([128, 2 * INNER], F32, tag=f"wq{kt}")
                nc.sync.dma_start(out=wk[:, 0:512],
                                  in_=wqk[kt * 128:(kt + 1) * 128, 0:512])
                nc.sync.dma_start(out=wk[:, 512:1024],
                                  in_=wqk[kt * 128:(kt + 1) * 128, 512:1024])
                xt_t.append(xk)
                wq_t.append(wk)
            xtb = cst.tile([128, 4, N], BF16)
            nc.sync.dma_start(out=xtb, in_=xTbf[:, :].rearrange("(t p) i -> p t i", p=128))
            msk = cst.tile([128, 8, N], BF16)
            nc.sync.dma_start(out=msk, in_=maskT[:, :].rearrange("(t p) i -> p t i", p=128))
            wvb = cst.tile([128, 4, INNER], BF16)
            nc.sync.dma_start(out=wvb, in_=wvbf[:, :].rearrange("(t p) c -> p t c", p=128))
            wob = cst.tile([128, 4, DIM], BF16)
            nc.sync.dma_start(out=wob, in_=wobf[:, :].rearrange("(t p) e -> p t e", p=128))
            bb = cst.tile([128, DIM], F32)
            nc.sync.dma_start(out=bb, in_=bout[0:1, :].to_broadcast([128, DIM]))

            ones32 = cst.tile([128, 1], F32)
            nc.vector.memset(ones32, 1.0)
            onesbf = cst.tile([128, 1], BF16)
            nc.vector.memset(onesbf, 1.0)
            onesr1 = cst.tile([1, 128], BF16)
            nc.vector.memset(onesr1, 1.0)
            sel0 = cst.tile([128, 1], F32)
            nc.vector.memset(sel0, 0.0)
            nc.vector.memset(sel0[0:1, :], 1.0)
            nc.vector.memset(sel0[32:33, :], 1.0)
            sel1 = cst.tile([128, 1], F32)
            nc.vector.memset(sel1, 0.0)
            nc.vector.memset(sel1[64:65, :], 1.0)
            nc.vector.memset(sel1[96:97, :], 1.0)

            qkT = cst.tile([128, 8, N], F32)
            vv = cst.tile([128, HEADS, 8, DH], BF16)
            onorm = cst.tile([128, 4, N], BF16)
            sc_acc = cst.tile([128, N], F32)
            nc.vector.memset(sc_acc, 0.0)

            # ---- QKV q/k part (fp32) ----
            for ct in range(8):
                for ic in range(2):
                    qtag = "dA" if (ct * 2 + ic) % 2 == 0 else "dB"
                    qpool = ppA if qtag == "dA" else ppB
                    pq = qpool.tile([128, N], F32, tag=qtag)
                    for kt in range(4):
                        nc.tensor.matmul(
                            pq[:, ic * 512:(ic + 1) * 512],
                            wq_t[kt][:, ct * 128:(ct + 1) * 128],
                            xt_t[kt][:, ic * 512:(ic + 1) * 512],
                            start=(kt == 0), stop=(kt == 3))
                    nc.vector.tensor_copy(qkT[:, ct, ic * 512:(ic + 1) * 512],
                                          pq[:, ic * 512:(ic + 1) * 512])

            # ---- V part (bf16) ----
            for jt in range(8):
                vpool, vtag = (ppB, "dB") if jt % 2 == 0 else (ppA, "dA")
                pv = vpool.tile([128, N], F32, tag=vtag)
                for kt in range(4):
                    nc.tensor.matmul(
                        pv[:, 0:512],
                        xtb[:, kt, jt * 128:(jt + 1) * 128],
                        wvb[:, kt, :],
                        start=(kt == 0), stop=(kt == 3))
                nc.vector.tensor_copy(
                    vv[:, :, jt, :],
                    pv[:, 0:512].rearrange("p (h d) -> p h d", h=HEADS))

            # ---- nnz (needs only msk): compute early, off the tail ----
            nzp = ppB.tile([1, N], F32, tag="dB")
            for jt in range(8):
                for ic in range(2):
                    sl = slice(ic * 512, (ic + 1) * 512)
                    nc.tensor.matmul(nzp[0:1, sl], onesbf, msk[:, jt, sl],
                                     start=(jt == 0), stop=(jt == 7))
            scr = one.tile([1, N], F32, tag="scr")
            rnz = one.tile([1, N], F32, tag="rnz")
            nc.vector.reciprocal_approx_accurate(out=rnz, in_=nzp, scratch=scr)

            # ---- attention, head pairs ----
            for pr in range(4):
                hA, hB = 2 * pr, 2 * pr + 1
                oTp = poT.tile([128, N], F32, tag="oT")
                pack = ppk.tile([128, N], F32, tag="pk")
                for jt in range(8):
                    dA = ppA.tile([128, N], F32, tag="dA")
                    dB = ppB.tile([128, N], F32, tag="dB")
                    for ic in range(2):
                        nc.tensor.matmul(
                            dA[:, ic * 512:(ic + 1) * 512],
                            qkT[0:64, 4 + pr, jt * 128:(jt + 1) * 128],
                            qkT[0:64, pr, ic * 512:(ic + 1) * 512],
                            start=True, stop=True, tile_position=(0, 0))
                        nc.tensor.matmul(
                            dB[:, ic * 512:(ic + 1) * 512],
                            qkT[64:128, 4 + pr, jt * 128:(jt + 1) * 128],
                            qkT[64:128, pr, ic * 512:(ic + 1) * 512],
                            start=True, stop=True, tile_position=(64, 0))
                    es, abs_ = [None, None], [None, None]
                    order = ((0, dA), (1, dB)) if jt % 2 == 0 else ((1, dB), (0, dA))
                    for hh, dots in order:
                        t = wrk4.tile([128, N], F32, tag="t")
                        nc.vector.tensor_tensor(out=t, in0=dots, in1=msk[:, jt, :],
                                                op=AOP.mult)
                        e = wrk4.tile([128, N], BF16, tag="e")
                        nc.scalar.activation(out=e, in_=t, func=AFT.Exp, scale=SCALE)
                        ab = wrk4.tile([128, N], F32, tag="ab")
                        nc.vector.tensor_scalar(
                            out=ab.bitcast(U32), in0=t.bitcast(U32),
                            scalar1=0x7FFFFFFF, scalar2=None, op0=AOP.bitwise_and)
                        es[hh] = e
                        abs_[hh] = ab
                    # matmul burst: adjacent same-kind MMs land on distinct
                    # strips so the PE runs them concurrently
                    first, last = (jt == 0), (jt == 7)
                    for ic in range(2):
                        sl = slice(ic * 512, (ic + 1) * 512)
                        for hh in range(2):
                            nc.tensor.matmul(
                                oTp[hh * 64:(hh + 1) * 64, sl],
                                vv[:, 2 * pr + hh, jt, :], es[hh][:, sl],
                                start=first, stop=last,
                                tile_position=(0, hh * 64),
                                skip_group_check=True)
                    # all four fp32 score MMs concurrent on strips 0-3
                    for ic in range(2):
                        sl = slice(ic * 512, (ic + 1) * 512)
                        for hh in range(2):
                            st = hh * 32 + ic * 64
                            nc.tensor.matmul(
                                pack[st:st + 1, sl],
                                ones32, abs_[hh][:, sl],
                                start=first, stop=last,
                                tile_position=(0, st),
                                skip_group_check=True)
                    # four bf16 Z MMs in the complementary cells
                    for ic in range(2):
                        sl = slice(ic * 512, (ic + 1) * 512)
                        for hh in range(2):
                            st = hh * 32 + (1 - ic) * 64
                            nc.tensor.matmul(
                                pack[st:st + 1, sl],
                                onesbf, es[hh][:, sl],
                                start=first, stop=last,
                                tile_position=(0, st),
                                skip_group_check=True)
                # harvest: score cells r0/r32 (ic0) + r64/r96 (ic1) -> sc_acc
                nc.vector.tensor_tensor(out=sc_acc[0:97, :], in0=sc_acc[0:97, :],
                                        in1=pack[0:97, :], op=AOP.add)
                # Z_A = {row64 ic0, row0 ic1}; Z_B = {row96 ic0, row32 ic1}
                zshift = eph.tile([128, 2, N], BF16, tag="zsh")
                zrow = eph.tile([1, 2, N], BF16, tag="zrow")
                nc.scalar.activation(out=zshift[64:65, 0, 0:512],
                                     in_=pack[64:65, 0:512], func=AFT.Copy)
                nc.scalar.activation(out=zrow[0:1, 0, 512:1024],
                                     in_=pack[0:1, 512:1024], func=AFT.Copy)
                nc.scalar.activation(out=zshift[96:97, 1, 0:512],
                                     in_=pack[96:97, 0:512], func=AFT.Copy)
                nc.scalar.activation(out=zshift[32:33, 1, 512:1024],
                                     in_=pack[32:33, 512:1024], func=AFT.Copy)
                # partition shifts to row 0 via SBUF->SBUF DMA
                nc.sync.dma_start(out=zrow[0:1, 0, 0:512], in_=zshift[64:65, 0, 0:512])
                nc.sync.dma_start(out=zrow[0:1, 1, 0:512], in_=zshift[96:97, 1, 0:512])
                nc.sync.dma_start(out=zrow[0:1, 1, 512:1024],
                                  in_=zshift[32:33, 1, 512:1024])
                # broadcast Z over partitions: rows 0:64 = Z_A, 64:128 = Z_B
                zbc = ppk.tile([128, N], F32, tag="pk")
                for ic in range(2):
                    sl = slice(ic * 512, (ic + 1) * 512)
                    nc.tensor.matmul(zbc[0:64, sl], onesr1[:, 0:64],
                                     zrow[0:1, 0, sl],
                                     start=True, stop=True, tile_position=(0, 0))
                    nc.tensor.matmul(zbc[64:128, sl], onesr1[:, 0:64],
                                     zrow[0:1, 1, sl],
                                     start=True, stop=True, tile_position=(0, 64))
                zr = eph.tile([128, N], F32, tag="zr")
                nc.vector.reciprocal_approx_fast(out=zr, in_=zbc)
                nc.vector.tensor_tensor(out=onorm[:, pr, :], in0=oTp, in1=zr,
                                        op=AOP.mult)
                if pr == 3:
                    scp = ppB.tile([1, N], F32, tag="dB")
                    nc.tensor.matmul(scp[0:1, 0:512], sel0, sc_acc[:, 0:512],
                                     start=True, stop=True)
                    nc.tensor.matmul(scp[0:1, 512:1024], sel1,
                                     sc_acc[:, 512:1024], start=True, stop=True)

            # ---- output projection (per pair, K=128) ----
            for it in range(8):
                ypool, ytag = (ppA, "dA") if it % 2 == 0 else (ppB, "dB")
                yp = ypool.tile([128, N], F32, tag=ytag)
                for pr in range(4):
                    nc.tensor.matmul(
                        yp[:, 0:512],
                        onorm[:, pr, it * 128:(it + 1) * 128],
                        wob[:, pr, :],
                        start=(pr == 0), stop=(pr == 3))
                yt = eph.tile([128, DIM], F32, tag="yt")
                nc.vector.tensor_tensor(out=yt, in0=yp[:, 0:512], in1=bb, op=AOP.add)
                nc.sync.dma_start(out=y_out[it * 128:(it + 1) * 128, :], in_=yt)

            # ---- score: sum the 8 per-head rows, / nnz, * scale ----
            sc_sb = one.tile([1, N], F32, tag="scs")
            nc.vector.scalar_tensor_tensor(
                out=sc_sb, in0=scp, scalar=SCALE, in1=rnz,
                op0=AOP.mult, op1=AOP.mult)

            # ---- outputs ----
            nc.gpsimd.dma_start(out=sc_out[:, :], in_=sc_sb)
    nc.finalize()
    return nc


def _get_nc():
    if "nc" not in _cache:
        _cache["nc"] = _build()
    return _cache["nc"]


def _run_device(inputs, trace=False):
    x = np.asarray(inputs["x"], np.float32)
    cp_mask = np.asarray(inputs["cp_mask"])
    w_qkv = np.asarray(inputs["w_qkv"], np.float32)
    w_out = np.asarray(inputs["w_out"], np.float32)
    b_out = np.asarray(inputs["b_out"], np.float32)

    bf = mybir.dt.np(BF16)
    maskT = np.ascontiguousarray(cp_mask.T).astype(bf)
    wqk = np.ascontiguousarray(w_qkv[:, :2 * INNER])
    wvbf = np.ascontiguousarray(w_qkv[:, 2 * INNER:]).astype(bf)
    wobf = np.ascontiguousarray(w_out).astype(bf)
    boutr = np.ascontiguousarray(b_out.reshape(1, DIM))

    in_maps = []
    for b in range(B):
        xTb = np.ascontiguousarray(x[b].T)
        in_maps.append({
            "xT": xTb,
            "xTbf": xTb.astype(bf),
            "maskT": maskT,
            "wqk": wqk,
            "wvbf": wvbf,
            "wobf": wobf,
            "bout": boutr,
        })

    nc = _get_nc()
    res = run_bass_kernel_spmd(nc, in_maps, core_ids=list(range(B)), trace=trace)
    y = np.stack([res.results[b]["y"] for b in range(B)])
    score = np.stack([res.results[b]["score"][0] for b in range(B)])
    return y, score, res


def _apply_swap(y, score, patches):
    idx = np.argsort(score, axis=-1, kind="stable")[::-1]
    out = y.copy()
    clone = y
    bi = np.arange(B)
    for i in range(1, patches + 1):
        ti = idx[:, i]
        out[bi, i] = clone[bi, ti]
        out[bi, ti] = clone[:, i]
    return out


def kernel(**inputs):
    patches = int(np.asarray(inputs["patches_in_core_nodes"]))
    y, score, _ = _run_device(inputs, trace=False)
    return _apply_swap(y, score, patches)


# revision 21
# speedup vs baseline: 1.1675x; 1.1675x over previous
"""CPAttention Trainium2 kernel: 8-way batch-data-parallel over 8 NeuronCores.

v2: head-pair processing with PE packing.
  - dots: fp32, two heads row-packed (K=64 at tile_position (0,0)/(64,0))
  - AV:   bf16, two heads col-packed into one [128,1024] PSUM (cols 0:64/64:128)
  - pack: 4-col-packed ones-matmuls -> score_A(row0, fp32), score_B(row32, fp32),
          Z_A(row64, bf16), Z_B(row96, bf16), accumulated over j-tiles
  - outproj: per-pair K=128 bf16
Score path (argsort-critical) stays fp32; softmax/output path is bf16.
Host applies the argsort + 16-step row swap (commutes with w_out).
"""
import numpy as np

import concourse.bacc as bacc
import concourse.tile as tile
from concourse import mybir
from concourse.bass_utils import run_bass_kernel_spmd

F32 = mybir.dt.float32
BF16 = mybir.dt.bfloat16
U32 = mybir.dt.uint32
AOP = mybir.AluOpType
AFT = mybir.ActivationFunctionType

B, N, DIM = 8, 1024, 512
HEADS, DH = 8, 64
INNER = 512
SCALE = DH ** -0.5

_cache = {}


def _build():
    nc = bacc.Bacc()
    xT = nc.declare_dram_parameter("xT", [DIM, N], F32, isOutput=False)
    xTbf = nc.declare_dram_parameter("xTbf", [DIM, N], BF16, isOutput=False)
    maskT = nc.declare_dram_parameter("maskT", [N, N], BF16, isOutput=False)
    wqk = nc.declare_dram_parameter("wqk", [DIM, 2 * INNER], F32, isOutput=False)
    wvbf = nc.declare_dram_parameter("wvbf", [DIM, INNER], BF16, isOutput=False)
    wobf = nc.declare_dram_parameter("wobf", [INNER, DIM], BF16, isOutput=False)
    bout = nc.declare_dram_parameter("bout", [1, DIM], F32, isOutput=False)
    y_out = nc.declare_dram_parameter("y", [N, DIM], F32, isOutput=True)
    sc_out = nc.declare_dram_parameter("score", [1, N], F32, isOutput=True)

    with tile.TileContext(nc) as tc:
        with tc.tile_pool(name="cst", bufs=1) as cst, \
             tc.tile_pool(name="wrk", bufs=3) as wrk, \
             tc.tile_pool(name="wrk4", bufs=4) as wrk4, \
             tc.tile_pool(name="eph", bufs=2) as eph, \
             tc.tile_pool(name="one", bufs=1) as one, \
             tc.tile_pool(name="ppA", bufs=1, space="PSUM") as ppA, \
             tc.tile_pool(name="ppB", bufs=1, space="PSUM") as ppB, \
             tc.tile_pool(name="poT", bufs=1, space="PSUM") as poT, \
             tc.tile_pool(name="ppk", bufs=1, space="PSUM") as ppk:

            # ---- loads ----
            xt_t = []
            wq_t = []
            for kt in range(4):
                xk = cst.tile([128, N], F32, tag=f"xt{kt}")
                nc.sync.dma_start(out=xk, in_=xT[kt * 128:(kt + 1) * 128, :])
                wk = cst.tile([128, 2 * INNER], F32, tag=f"wq{kt}")
                nc.sync.dma_start(out=wk[:, 0:512],
                                  in_=wqk[kt * 128:(kt + 1) * 128, 0:512])
                nc.sync.dma_start(out=wk[:, 512:1024],
                                  in_=wqk[kt * 128:(kt + 1) * 128, 512:1024])
                xt_t.append(xk)
                wq_t.append(wk)
            xtb = cst.tile([128, 4, N], BF16)
            nc.sync.dma_start(out=xtb, in_=xTbf[:, :].rearrange("(t p) i -> p t i", p=128))
            msk = cst.tile([128, 8, N], BF16)
            nc.sync.dma_start(out=msk, in_=maskT[:, :].rearrange("(t p) i -> p t i", p=128))
            wvb = cst.tile([128, 4, INNER], BF16)
            nc.sync.dma_start(out=wvb, in_=wvbf[:, :].rearrange("(t p) c -> p t c", p=128))
            wob = cst.tile([128, 4, DIM], BF16)
            nc.sync.dma_start(out=wob, in_=wobf[:, :].rearrange("(t p) e -> p t e", p=128))
            bb = cst.tile([128, DIM], F32)
            nc.sync.dma_start(out=bb, in_=bout[0:1, :].to_broadcast([128, DIM]))

            ones32 = cst.tile([128, 1], F32)
            nc.vector.memset(ones32, 1.0)
            onesbf = cst.tile([128, 1], BF16)
            nc.vector.memset(onesbf, 1.0)
            onesr1 = cst.tile([1, 128], BF16)
            nc.vector.memset(onesr1, 1.0)
            sel0 = cst.tile([128, 1], F32)
            nc.vector.memset(sel0, 0.0)
            nc.vector.memset(sel0[0:1, :], 1.0)
            nc.vector.memset(sel0[32:33, :], 1.0)
            sel1 = cst.tile([128, 1], F32)
            nc.vector.memset(sel1, 0.0)
            nc.vector.memset(sel1[64:65, :], 1.0)
            nc.vector.memset(sel1[96:97, :], 1.0)

            qkT = cst.tile([128, 8, N], F32)
            vv = cst.tile([128, HEADS, 8, DH], BF16)
            onorm = cst.tile([128, 4, N], BF16)
            sc_acc = cst.tile([128, N], F32)
            nc.vector.memset(sc_acc, 0.0)

            # ---- QKV q/k part (fp32) ----
            for ct in range(8):
                for ic in range(2):
                    qtag = "dA" if (ct * 2 + ic) % 2 == 0 else "dB"
                    qpool = ppA if qtag == "dA" else ppB
                    pq = qpool.tile([128, N], F32, tag=qtag)
                    for kt in range(4):
                        nc.tensor.matmul(
                            pq[:, ic * 512:(ic + 1) * 512],
                            wq_t[kt][:, ct * 128:(ct + 1) * 128],
                            xt_t[kt][:, ic * 512:(ic + 1) * 512],
                            start=(kt == 0), stop=(kt == 3))
                    nc.vector.tensor_copy(qkT[:, ct, ic * 512:(ic + 1) * 512],
                                          pq[:, ic * 512:(ic + 1) * 512])

            # ---- V part (bf16) ----
            for jt in range(8):
                vpool, vtag = (ppB, "dB") if jt % 2 == 0 else (ppA, "dA")
                pv = vpool.tile([128, N], F32, tag=vtag)
                for kt in range(4):
                    nc.tensor.matmul(
                        pv[:, 0:512],
                        xtb[:, kt, jt * 128:(jt + 1) * 128],
                        wvb[:, kt, :],
                        start=(kt == 0), stop=(kt == 3))
                nc.vector.tensor_copy(
                    vv[:, :, jt, :],
                    pv[:, 0:512].rearrange("p (h d) -> p h d", h=HEADS))

            # ---- nnz (needs only msk): compute early, off the tail ----
            nzp = ppB.tile([1, N], F32, tag="dB")
            for jt in range(8):
                for ic in range(2):
                    sl = slice(ic * 512, (ic + 1) * 512)
                    nc.tensor.matmul(nzp[0:1, sl], onesbf, msk[:, jt, sl],
                                     start=(jt == 0), stop=(jt == 7))
            scr = one.tile([1, N], F32, tag="scr")
            rnz = one.tile([1, N], F32, tag="rnz")
            nc.vector.reciprocal_approx_accurate(out=rnz, in_=nzp, scratch=scr)

            # ---- attention, head pairs ----
            for pr in range(4):
                hA, hB = 2 * pr, 2 * pr + 1
                oTp = poT.tile([128, N], F32, tag="oT")
                pack = ppk.tile([128, N], F32, tag="pk")
                for jt in range(8):
                    dA = ppA.tile([128, N], F32, tag="dA")
                    dB = ppB.tile([128, N], F32, tag="dB")
                    for ic in range(2):
                        nc.tensor.matmul(
                            dA[:, ic * 512:(ic + 1) * 512],
                            qkT[0:64, 4 + pr, jt * 128:(jt + 1) * 128],
                            qkT[0:64, pr, ic * 512:(ic + 1) * 512],
                            start=True, stop=True, tile_position=(0, 0))
                        nc.tensor.matmul(
                            dB[:, ic * 512:(ic + 1) * 512],
                            qkT[64:128, 4 + pr, jt * 128:(jt + 1) * 128],
                            qkT[64:128, pr, ic * 512:(ic + 1) * 512],
                            start=True, stop=True, tile_position=(64, 0))
                    es, abs_ = [], []
                    for hh, dots in ((0, dA), (1, dB)):
                        t = wrk.tile([128, N], F32, tag="t")
                        nc.vector.tensor_tensor(out=t, in0=dots, in1=msk[:, jt, :],
                                                op=AOP.mult)
                        e = wrk4.tile([128, N], BF16, tag="e")
                        nc.scalar.activation(out=e, in_=t, func=AFT.Exp, scale=SCALE)
                        ab = wrk4.tile([128, N], F32, tag="ab")
                        nc.vector.tensor_scalar(
                            out=ab.bitcast(U32), in0=t.bitcast(U32),
                            scalar1=0x7FFFFFFF, scalar2=None, op0=AOP.bitwise_and)
                        es.append(e)
                        abs_.append(ab)
                    # matmul burst: adjacent same-kind MMs land on distinct
                    # strips so the PE runs them concurrently
                    first, last = (jt == 0), (jt == 7)
                    for ic in range(2):
                        sl = slice(ic * 512, (ic + 1) * 512)
                        for hh in range(2):
                            nc.tensor.matmul(
                                oTp[hh * 64:(hh + 1) * 64, sl],
                                vv[:, 2 * pr + hh, jt, :], es[hh][:, sl],
                                start=first, stop=last,
                                tile_position=(0, hh * 64),
                                skip_group_check=True)
                    # all four fp32 score MMs concurrent on strips 0-3
                    for ic in range(2):
                        sl = slice(ic * 512, (ic + 1) * 512)
                        for hh in range(2):
                            st = hh * 32 + ic * 64
                            nc.tensor.matmul(
                                pack[st:st + 1, sl],
                                ones32, abs_[hh][:, sl],
                                start=first, stop=last,
                                tile_position=(0, st),
                                skip_group_check=True)
                    # four bf16 Z MMs in the complementary cells
                    for ic in range(2):
                        sl = slice(ic * 512, (ic + 1) * 512)
                        for hh in range(2):
                            st = hh * 32 + (1 - ic) * 64
                            nc.tensor.matmul(
                                pack[st:st + 1, sl],
                                onesbf, es[hh][:, sl],
                                start=first, stop=last,
                                tile_position=(0, st),
                                skip_group_check=True)
                # harvest: score cells r0/r32 (ic0) + r64/r96 (ic1) -> sc_acc
                nc.vector.tensor_tensor(out=sc_acc[0:97, :], in0=sc_acc[0:97, :],
                                        in1=pack[0:97, :], op=AOP.add)
                # Z_A = {row64 ic0, row0 ic1}; Z_B = {row96 ic0, row32 ic1}
                zshift = eph.tile([128, 2, N], BF16, tag="zsh")
                zrow = eph.tile([1, 2, N], BF16, tag="zrow")
                nc.scalar.activation(out=zshift[64:65, 0, 0:512],
                                     in_=pack[64:65, 0:512], func=AFT.Copy)
                nc.scalar.activation(out=zrow[0:1, 0, 512:1024],
                                     in_=pack[0:1, 512:1024], func=AFT.Copy)
                nc.scalar.activation(out=zshift[96:97, 1, 0:512],
                                     in_=pack[96:97, 0:512], func=AFT.Copy)
                nc.scalar.activation(out=zshift[32:33, 1, 512:1024],
                                     in_=pack[32:33, 512:1024], func=AFT.Copy)
                # partition shifts to row 0 via SBUF->SBUF DMA
                nc.sync.dma_start(out=zrow[0:1, 0, 0:512], in_=zshift[64:65, 0, 0:512])
                nc.sync.dma_start(out=zrow[0:1, 1, 0:512], in_=zshift[96:97, 1, 0:512])
                nc.sync.dma_start(out=zrow[0:1, 1, 512:1024],
                                  in_=zshift[32:33, 1, 512:1024])
                # broadcast Z over partitions: rows 0:64 = Z_A, 64:128 = Z_B
                zbc = ppk.tile([128, N], F32, tag="pk")
                for ic in range(2):
                    sl = slice(ic * 512, (ic + 1) * 512)
                    nc.tensor.matmul(zbc[0:64, sl], onesr1[:, 0:64],
                                     zrow[0:1, 0, sl],
                                     start=True, stop=True, tile_position=(0, 0))
                    nc.tensor.matmul(zbc[64:128, sl], onesr1[:, 0:64],
                                     zrow[0:1, 1, sl],
                                     start=True, stop=True, tile_position=(0, 64))
                zr = eph.tile([128, N], F32, tag="zr")
                nc.vector.reciprocal_approx_fast(out=zr, in_=zbc)
                nc.vector.tensor_tensor(out=onorm[:, pr, :], in0=oTp, in1=zr,
                                        op=AOP.mult)
                if pr == 3:
                    scp = ppB.tile([1, N], F32, tag="dB")
                    nc.tensor.matmul(scp[0:1, 0:512], sel0, sc_acc[:, 0:512],
                                     start=True, stop=True)
                    nc.tensor.matmul(scp[0:1, 512:1024], sel1,
                                     sc_acc[:, 512:1024], start=True, stop=True)

            # ---- output projection (per pair, K=128) ----
            for it in range(8):
                ypool, ytag = (ppA, "dA") if it % 2 == 0 else (ppB, "dB")
                yp = ypool.tile([128, N], F32, tag=ytag)
                for pr in range(4):
                    nc.tensor.matmul(
                        yp[:, 0:512],
                        onorm[:, pr, it * 128:(it + 1) * 128],
                        wob[:, pr, :],
                        start=(pr == 0), stop=(pr == 3))
                yt = eph.tile([128, DIM], F32, tag="yt")
                nc.vector.tensor_tensor(out=yt, in0=yp[:, 0:512], in1=bb, op=AOP.add)
                nc.sync.dma_start(out=y_out[it * 128:(it + 1) * 128, :], in_=yt)

            # ---- score: sum the 8 per-head rows, / nnz, * scale ----
            sc_sb = one.tile([1, N], F32, tag="scs")
            nc.vector.scalar_tensor_tensor(
                out=sc_sb, in0=scp, scalar=SCALE, in1=rnz,
                op0=AOP.mult, op1=AOP.mult)

            # ---- outputs ----
            nc.gpsimd.dma_start(out=sc_out[:, :], in_=sc_sb)
    nc.finalize()
    return nc


def _get_nc():
    if "nc" not in _cache:
        _cache["nc"] = _build()
    return _cache["nc"]


def _run_device(inputs, trace=False):
    x = np.asarray(inputs["x"], np.float32)
    cp_mask = np.asarray(inputs["cp_mask"])
    w_qkv = np.asarray(inputs["w_qkv"], np.float32)
    w_out = np.asarray(inputs["w_out"], np.float32)
    b_out = np.asarray(inputs["b_out"], np.float32)

    bf = mybir.dt.np(BF16)
    maskT = np.ascontiguousarray(cp_mask.T).astype(bf)
    wqk = np.ascontiguousarray(w_qkv[:, :2 * INNER])
    wvbf = np.ascontiguousarray(w_qkv[:, 2 * INNER:]).astype(bf)
    wobf = np.ascontiguousarray(w_out).astype(bf)
    boutr = np.ascontiguousarray(b_out.reshape(1, DIM))

    in_maps = []
    for b in range(B):
        xTb = np.ascontiguousarray(x[b].T)
        in_maps.append({
            "xT": xTb,
            "xTbf": xTb.astype(bf),
            "maskT": maskT,
            "wqk": wqk,
            "wvbf": wvbf,
            "wobf": wobf,
            "bout": boutr,
        })

    nc = _get_nc()
    res = run_bass_kernel_spmd(nc, in_maps, core_ids=list(range(B)), trace=trace)
    y = np.stack([res.results[b]["y"] for b in range(B)])
    score = np.stack([res.results[b]["score"][0] for b in range(B)])
    return y, score, res


def _apply_swap(y, score, patches):
    idx = np.argsort(score, axis=-1, kind="stable")[::-1]
    out = y.copy()
    clone = y
    bi = np.arange(B)
    for i in range(1, patches + 1):
        ti = idx[:, i]
        out[bi, i] = clone[bi, ti]
        out[bi, ti] = clone[:, i]
    return out


def kernel(**inputs):
    patches = int(np.asarray(inputs["patches_in_core_nodes"]))
    y, score, _ = _run_device(inputs, trace=False)
    return _apply_swap(y, score, patches)
